# revision 2
# baseline (speedup 1.0000x reference)
"""Haar 2x2 stride-2 DWT kernel for TRN2 (8 NeuronCores, batch-parallel).

Input  x: [8, 96, 384, 384] f32.
Output: tuple of 4 identical arrays [8, 96, 192, 192] f32 (the reference's
filter index (g*C + c) % 4 == c % 4 is independent of group g since C % 4 == 0).

Per channel c, with a,b,c_,d the 2x2 taps (TL, TR, BL, BR):
  c%4==0 (ll):  0.5( a + b + c_ + d)
  c%4==1 (lh):  0.5(-a - b + c_ + d)
  c%4==2 (hl):  0.5(-a + b - c_ + d)
  c%4==3 (hh):  0.5( a - b - c_ + d)
Separable form used here (per-partition sign scalars sv, sc):
  V = sv*top_row + bot_row          (vertical,  sv = -1 iff c%4 in {1,3})
  T = sc*V_even + V_odd             (horizontal, sc = -1 iff c%4 in {2,3})
  out = 0.5 * T
"""

import sys

sys.path.insert(0, "/opt/trn_rl_repo")

import numpy as np

import concourse.bacc as bacc
import concourse.bass as bass
import concourse.mybir as mybir
import concourse.tile as tile
from concourse.bass_utils import run_bass_kernel_spmd

F32 = mybir.dt.float32

N, C, H, W = 8, 96, 384, 384
H2, W2 = H // 2, W // 2
N_CORES = 8
# Per core: C*H = 36864 image rows -> 18432 vertical pairs.
PAIRS = C * H // 2
PP = 4  # pairs per partition per tile
PART = 128
PAIRS_PER_TILE = PART * PP  # 512
NT = PAIRS // PAIRS_PER_TILE  # 36
PAIRS_PER_CH = H // 2  # 192


def _sign_array() -> np.ndarray:
    """[128, 6] f32: columns (2*(t%3), 2*(t%3)+1) = (sv, sc) for tile t."""
    signs = np.empty((PART, 6), dtype=np.float32)
    for t in range(3):
        for p in range(PART):
            pair = PAIRS_PER_TILE * t + PP * p
            m = (pair // PAIRS_PER_CH) % 4
            signs[p, 2 * t] = -1.0 if m in (1, 3) else 1.0
            signs[p, 2 * t + 1] = -1.0 if m in (2, 3) else 1.0
    return signs


def _build():
    nc = bacc.Bacc("TRN2", target_bir_lowering=False, debug=False,
                   num_devices=N_CORES)
    x = nc.dram_tensor("x", [NT, PART, PP, 2, W], F32, kind="ExternalInput")
    signs = nc.dram_tensor("signs", [PART, 6], F32, kind="ExternalInput")
    out = nc.dram_tensor("out", [NT, PART, PP, W2], F32, kind="ExternalOutput")

    with tile.TileContext(nc) as tc:
        with tc.tile_pool(name="const", bufs=1) as cpool, \
             tc.tile_pool(name="inp", bufs=3) as ipool, \
             tc.tile_pool(name="vert", bufs=3) as vpool, \
             tc.tile_pool(name="horz", bufs=3) as hpool, \
             tc.tile_pool(name="outp", bufs=3) as opool:
            sgn = cpool.tile([PART, 6], F32)
            nc.sync.dma_start(out=sgn[:, :], in_=signs[:, :])

            for t in range(NT):
                k = t % 3
                sv = sgn[:, 2 * k:2 * k + 1]
                sc = sgn[:, 2 * k + 1:2 * k + 2]

                tin = ipool.tile([PART, PP, 2, W], F32)
                nc.sync.dma_start(out=tin[:, :, :, :], in_=x[t])

                v = vpool.tile([PART, PP, W], F32)
                nc.vector.scalar_tensor_tensor(
                    out=v[:, :, :],
                    in0=tin[:, :, 0, :],
                    scalar=sv,
                    in1=tin[:, :, 1, :],
                    op0=mybir.AluOpType.mult,
                    op1=mybir.AluOpType.add,
                )

                th = hpool.tile([PART, PP, W2], F32)
                nc.vector.scalar_tensor_tensor(
                    out=th[:, :, :],
                    in0=v[:, :, 0::2],
                    scalar=sc,
                    in1=v[:, :, 1::2],
                    op0=mybir.AluOpType.mult,
                    op1=mybir.AluOpType.add,
                )

                o = opool.tile([PART, PP, W2], F32)
                nc.scalar.mul(o[:, :, :], th[:, :, :], 0.5)

                nc.sync.dma_start(out=out[t], in_=o[:, :, :])

    nc.compile()
    return nc


_NC = None


def _get_nc():
    global _NC
    if _NC is None:
        _NC = _build()
    return _NC


def kernel(x: np.ndarray):
    assert x.shape == (N, C, H, W) and x.dtype == np.float32
    nc = _get_nc()
    signs = _sign_array()
    in_maps = [
        {"x": np.ascontiguousarray(x[i]).reshape(NT, PART, PP, 2, W),
         "signs": signs}
        for i in range(N_CORES)
    ]
    res = run_bass_kernel_spmd(nc, in_maps, list(range(N_CORES)))
    full = np.stack(
        [res.results[i]["out"].reshape(C, H2, W2) for i in range(N_CORES)]
    )
    return (full, full, full, full)
